# revision 11
# baseline (speedup 1.0000x reference)
"""Cross-attention kernel for Trainium2, distributed over 8 NeuronCores.

Problem: B=4, Sk=4096, Sq=2048, d_model=1024, dims=64 (fp32 reference).

Sharding (hardcoded): core c -> (batch b = c//2, ENCODER half kh = c%2).
Each core computes partial-softmax attention of ALL 2048 decoder rows of its
batch against its 2048-key half of the encoder: a numerator [64, 2048] and a
denominator row accumulated in the same PSUM tile via a ones-column in the AV
lhsT. The host merges the two halves ((num0+num1)/(den0+den1)) and
transposes -- so the device does no softmax normalization, no output
transposes, and no collectives.

All compute is bf16: fp8 anywhere in the score/weight path (tested: at-fp8
alone 2.6e-2, v-fp8 1.9e-2, enc-fp8 4e-2 vs the 2e-2 gate) fails numerics,
so the PE floor is ~100k cycles/core and the kernel is PE-bound. The wins
over the original baseline are scheduling:
  - Flat 32-step software pipeline across both decoder halves (S of step g+1
    issues before AV of step g, across the qh boundary too).
  - DMA: wq/wkv issue from the Scalar queue in parallel with Sync; only the
    four first-needed transfers (enc0/dec0 halves) go on Sync up front; the
    six later activation chunks issue from GpSimd (SWDGE) after its memsets,
    so the critical first 2.5 MB gets the full ~390 GB/s instead of
    fair-sharing with 6 MB of later traffic.
  - gpsimd memset order: scratch (PE warmup input) first so warmup matmuls
    start right after the preamble barrier and the PE HAM clock-gate
    releases just as real work arrives; the big kTd/vTx memsets run on the
    (otherwise idle until ~13us) Vector engine instead of gpsimd.
  - Prologue projections consume enc0/dec0 half-by-half as they land; the
    kTd chunk-0 evacuation runs on ACT while qTd's runs on DVE in parallel.
  - exp's 1/sqrt(dims) score scale is folded into the ACT free affine
    (scale=0.125), so weights stream unscaled.
  - The softmax num/den go back to DRAM as bf16 (half the store tail); the
    host divides in fp32.
"""

import numpy as np
import ml_dtypes

import concourse.bass as bass
import concourse.bacc as bacc
import concourse.tile as tile
from concourse import mybir
from concourse._compat import with_exitstack
from concourse.bass_utils import run_bass_kernel_spmd
from concourse.masks import make_identity

BF16 = mybir.dt.bfloat16
F32 = mybir.dt.float32
B, SK, SQ, D, DIMS = 4, 4096, 2048, 1024, 64
N_CORES = 8
SKC = SK // 2   # 2048 encoder keys per core
SQC = SQ        # full decoder per core
DC = D // 128   # 8 d_model chunks
KB = SKC // 128  # 16 k blocks
NCK = SKC // 512  # 4 kv chunks
N_WARM = 8
EXP_SCALE = float(DIMS) ** -0.5  # 0.125, applied in the ACT free affine


@with_exitstack
def _body(ctx, tc, encT, decT, wkv, out):
    nc = tc.nc

    singles = ctx.enter_context(tc.tile_pool(name="singles", bufs=1))
    loads = ctx.enter_context(tc.tile_pool(name="loads", bufs=1))
    pss_pool = ctx.enter_context(tc.tile_pool(name="pss", bufs=2, space="PSUM"))
    po_pool = ctx.enter_context(tc.tile_pool(name="po", bufs=1, space="PSUM"))
    aux_pool = ctx.enter_context(tc.tile_pool(name="aux", bufs=2, space="PSUM"))
    at_pool = ctx.enter_context(tc.tile_pool(name="at", bufs=3))

    # --- scratch memset FIRST on gpsimd: it gates the PE warmup matmuls ---
    scratch = singles.tile([128, 512], BF16)
    nc.gpsimd.memset(scratch, 0.0)

    # --- weights on the Scalar queue: wq (first consumer) then wkv ---
    w_sb = singles.tile([128, 2 * DC, 128], BF16)
    nc.scalar.dma_start(out=w_sb[:, DC:2 * DC, :], in_=wkv[:, DC:2 * DC, :])
    nc.scalar.dma_start(out=w_sb[:, 0:DC, :], in_=wkv[:, 0:DC, :])
    wkv_sb = w_sb[:, 0:DC, :]
    wq_sb = w_sb[:, DC:2 * DC, :]

    # --- activation tiles ---
    esb = [
        loads.tile([128, DC, 512], BF16, tag=f"esb{ck}", name=f"esb{ck}")
        for ck in range(NCK)
    ]
    dsb = [
        loads.tile([128, DC, 512], BF16, tag=f"dsb{qg}", name=f"dsb{qg}")
        for qg in range(4)
    ]
    enc_r = encT  # [8, 128, 4, 512] pre-chunked on host
    dec_r = decT

    def load_enc(eng, ck):
        eng.dma_start(
            out=esb[ck].rearrange("p (h c) n -> p h c n", h=2),
            in_=enc_r[2 * ck:2 * ck + 2].rearrange("h p c n -> p h c n"),
        )

    def load_dec(eng, qg):
        eng.dma_start(
            out=dsb[qg].rearrange("p (h c) n -> p h c n", h=2),
            in_=dec_r[2 * qg:2 * qg + 2].rearrange("h p c n -> p h c n"),
        )

    # first-needed transfers on Sync, half-granular, arrival-need order
    nc.sync.dma_start(out=esb[0][:, 0:4, :], in_=enc_r[0])
    nc.sync.dma_start(out=dsb[0][:, 0:4, :], in_=dec_r[0])
    nc.sync.dma_start(out=esb[0][:, 4:8, :], in_=enc_r[1])
    nc.sync.dma_start(out=dsb[0][:, 4:8, :], in_=dec_r[1])

    # --- small constants on gpsimd, then the six later chunks via SWDGE so
    # they don't steal bandwidth from the four critical transfers above ---
    bv_sb = singles.tile([DIMS, 1], F32)
    nc.gpsimd.memset(bv_sb, 0.0)
    bk_sb = singles.tile([DIMS, 1], F32)
    nc.gpsimd.memset(bk_sb, 0.0)
    bq_sb = singles.tile([128, 1], F32)
    nc.gpsimd.memset(bq_sb, 0.0)
    ident_bf = singles.tile([128, 128], BF16)
    make_identity(nc, ident_bf)

    gate = singles.tile([1, 1], BF16)

    # --- persistent activations. The big zero/ones fills run on Vector
    # (idle until the first evacuation at ~13us); kTd rows 64:128 stay ZERO
    # so S matmuls run a full K=128 contraction ---
    kTd = singles.tile([128, SKC], BF16)
    nc.vector.memset(kTd[DIMS:128, :], 0.0)
    vTx = singles.tile([DIMS + 1, SKC], BF16)  # V^T (ones come via vnat)
    vnat = singles.tile([128, KB, 80], BF16)   # V natural + ones col 64
    nc.vector.memset(vnat[:, :, DIMS:DIMS + 1], 1.0)
    qTd = singles.tile([128, SQC], BF16)  # Q^T (unscaled) duplicated

    # --- PE warmup during the DMA prologue (HAM clock-gate release) ---
    wm = pss_pool.tile([128, 2, 512], F32, tag="pss", name="pss_w")
    for i in range(N_WARM):
        nc.tensor.matmul(
            wm[:, i % 2, :], lhsT=scratch[:, 0:128], rhs=scratch,
            start=True, stop=True,
        )

    # --- K/V projection per 512-column chunk: lhsT = [Wv | Wk], rhs = encT
    # chunk -> psum [128, 512], rows 0:64 = V^T, 64:128 = K^T ---
    kv_ps = {}

    def kv_mms(ck, lo, hi):
        if ck not in kv_ps:
            kv_ps[ck] = aux_pool.tile(
                [128, 512], F32, tag="aux", name=f"pskv{ck % 2}")
        for d in range(lo, hi):
            nc.tensor.matmul(
                kv_ps[ck], lhsT=wkv_sb[:, d, :], rhs=esb[ck][:, d, :],
                start=(d == 0), stop=(d == DC - 1),
            )

    def kv_evac(ck):
        # kTd first: it alone gates the next S pair (and thus the exp chain)
        pskv = kv_ps.pop(ck)
        sl = slice(ck * 512, (ck + 1) * 512)
        nc.vector.tensor_scalar_add(kTd[0:DIMS, sl], pskv[DIMS:128, :], bk_sb)
        nc.vector.tensor_scalar_add(vTx[0:DIMS, sl], pskv[0:DIMS, :], bv_sb)

    def kv_tr(ck, half):
        for kb in range(ck * 4 + 2 * half, ck * 4 + 2 * half + 2):
            ptv = aux_pool.tile([128, 80], BF16, tag="aux", name=f"ptv{kb % 2}")
            nc.tensor.transpose(
                ptv[:, 0:DIMS], vTx[0:DIMS, kb * 128:(kb + 1) * 128],
                ident_bf[0:DIMS, 0:DIMS],
            )
            nc.vector.tensor_copy(vnat[:, kb, 0:DIMS], ptv[:, 0:DIMS])

    def kv_tr_dma(ck):
        # chunks with >=2 steps of slack transpose via the (idle) Sync DMA
        # xbar instead of the PE: ~1.8us latency each, zero PE cycles
        for kb in range(4 * ck, 4 * ck + 4):
            nc.sync.dma_start_transpose(
                out=vnat[:, kb, 0:DIMS],
                in_=vTx[0:DIMS, kb * 128:(kb + 1) * 128],
            )

    qp_ps = {}

    def qproj_mms(qg, lo, hi):
        if qg not in qp_ps:
            qp_ps[qg] = aux_pool.tile(
                [128, 512], F32, tag="aux", name=f"psq{qg % 2}")
        for d in range(lo, hi):
            nc.tensor.matmul(
                qp_ps[qg], lhsT=wq_sb[:, d, :], rhs=dsb[qg][:, d, :],
                start=(d == 0), stop=(d == DC - 1),
            )

    def qproj_evac(qg):
        psq = qp_ps.pop(qg)
        nc.vector.tensor_scalar_add(qTd[:, qg * 512:(qg + 1) * 512], psq, bq_sb)

    # --- flat 32-step pipeline: step g -> (qh, kbp, sub) ---
    steps_local = [
        (2 * g + i, s)
        for g in range(KB // 4) for s in range(2) for i in range(2)
    ]
    NSTEP = 32
    at_tiles = {}

    def s_and_exp(gq):
        qh, (kbp, sub) = gq // 16, steps_local[gq % 16]
        pss = pss_pool.tile([128, 2, 512], F32, tag="pss", name=f"pss{gq % 2}")
        q0 = qh * 1024 + sub * 512
        for i in range(2):
            kb = 2 * kbp + i
            nc.tensor.matmul(
                pss[:, i, :], lhsT=kTd[:, kb * 128:(kb + 1) * 128],
                rhs=qTd[:, q0:q0 + 512],
                start=True, stop=True,
            )
        at = at_pool.tile([128, 2, 512], BF16, tag="at", name=f"at{gq % 3}")
        at_tiles[gq] = at
        nc.scalar.activation(
            at.rearrange("p a n -> p (a n)"),
            pss.rearrange("p a n -> p (a n)"),
            mybir.ActivationFunctionType.Exp,
            scale=EXP_SCALE,
        )

    def av(gq, po):
        qh, (kbp, sub) = gq // 16, steps_local[gq % 16]
        at = at_tiles.pop(gq)
        for i in range(2):
            nc.tensor.matmul(
                po[:, sub, :], lhsT=vnat[:, 2 * kbp + i, 0:DIMS + 1],
                rhs=at[:, i, :],
                start=(kbp == 0 and i == 0),
                stop=(kbp == KB // 2 - 1 and i == 1),
            )

    # --- prologue compute: projections of chunk 0 in DMA-half order; kTd
    # evac on the (idle) ACT engine, qTd evac on DVE in parallel ---
    ident_fn = mybir.ActivationFunctionType.Identity
    kv_mms(0, 0, 4)
    qproj_mms(0, 0, 4)
    kv_mms(0, 4, 8)
    qproj_mms(0, 4, 8)
    pskv0 = kv_ps.pop(0)
    # DMA deferral gate: this copy depends on the tail of the enc0 transfer,
    # so gpsimd's six SWDGE issues below cannot start stealing HBM bandwidth
    # from the four critical transfers (GPSIMD cannot read PSUM, so gate on
    # the SBUF landing tile rather than the projection psum).
    nc.gpsimd.tensor_copy(gate, esb[0][0:1, 7, 511:512])
    load_dec(nc.gpsimd, 1)
    load_enc(nc.gpsimd, 1)
    load_enc(nc.gpsimd, 2)
    load_enc(nc.gpsimd, 3)
    load_dec(nc.gpsimd, 2)
    load_dec(nc.gpsimd, 3)
    nc.scalar.activation(kTd[0:DIMS, 0:512], pskv0[DIMS:128, :], ident_fn,
                         bias=bk_sb)
    psq0 = qp_ps.pop(0)
    nc.vector.tensor_scalar_add(qTd[:, 0:512], psq0, bq_sb)
    nc.vector.tensor_scalar_add(vTx[0:DIMS, 0:512], pskv0[0:DIMS, :], bv_sb)

    # --- extra PE work injected at the steps its DMA has landed. kv chunk c
    # must be evacuated before the S-pair issue for step 4c (one-ahead at
    # step 4c-1, after that step's extras); vnat block pair from kv_tr(c, h)
    # before the AV of the step that consumes it. ---
    extras = {
        0: lambda: (kv_tr(0, 0), qproj_mms(1, 0, 4)),
        1: lambda: (kv_tr(0, 1), qproj_mms(1, 4, 8), qproj_evac(1)),
        2: lambda: kv_mms(1, 0, 4),
        3: lambda: (kv_mms(1, 4, 8), kv_evac(1)),
        4: lambda: kv_tr(1, 0),
        5: lambda: (kv_tr(1, 1), kv_mms(2, 0, 4)),
        6: lambda: (kv_mms(2, 4, 8), kv_evac(2), kv_tr_dma(2)),
        9: lambda: kv_mms(3, 0, 4),
        10: lambda: (kv_mms(3, 4, 8), kv_evac(3), kv_tr_dma(3)),
        13: lambda: qproj_mms(2, 0, 4),
        14: lambda: (qproj_mms(2, 4, 8), qproj_evac(2)),
        16: lambda: qproj_mms(3, 0, 4),
        17: lambda: (qproj_mms(3, 4, 8), qproj_evac(3)),
    }
    out_r = out.rearrange("p (h s n) -> p h s n", h=2, s=2)
    oT = singles.tile([DIMS + 1, SQC], BF16)
    oT_r = oT.rearrange("p (h s n) -> p h s n", h=2, s=2)

    pos = {}
    for gq in range(NSTEP):
        qh = gq // 16
        kbp, sub = steps_local[gq % 16]
        if gq == 0:
            s_and_exp(0)
        if gq in extras:
            extras[gq]()
        if gq + 1 < NSTEP:
            s_and_exp(gq + 1)
        if qh not in pos:
            pos[qh] = po_pool.tile([DIMS + 1, 2, 512], F32, tag="po", name="po")
        av(gq, pos[qh])
        if (kbp, sub) == (KB // 2 - 1, 0):
            # sub 0's accumulation completes two steps before sub 1's:
            # evacuate + store it under the remaining steps
            nc.vector.tensor_copy(oT_r[:, qh, 0, :], pos[qh][:, 0, :])
            nc.sync.dma_start(out=out_r[:, qh, 0, :], in_=oT_r[:, qh, 0, :])
        elif (kbp, sub) == (KB // 2 - 1, 1):
            nc.vector.tensor_copy(oT_r[:, qh, 1, :], pos[qh][:, 1, :])
            nc.sync.dma_start(out=out_r[:, qh, 1, :], in_=oT_r[:, qh, 1, :])


_NC_CACHE = None


def _build():
    global _NC_CACHE
    if _NC_CACHE is not None:
        return _NC_CACHE
    nc = bacc.Bacc(
        "TRN2", target_bir_lowering=False, debug=False,
        enable_asserts=True, num_devices=N_CORES,
    )
    encT = nc.dram_tensor(
        "encT", [2 * NCK, 128, 4, 512], BF16, kind="ExternalInput").ap()
    decT = nc.dram_tensor(
        "decT", [2 * 4, 128, 4, 512], BF16, kind="ExternalInput").ap()
    wkv = nc.dram_tensor(
        "wkv", [128, 2 * DC, 128], BF16, kind="ExternalInput").ap()
    out = nc.dram_tensor("out", [DIMS + 1, SQC], BF16, kind="ExternalOutput").ap()
    with tile.TileContext(nc) as tc:
        _body(tc, encT, decT, wkv, out)
    nc.compile()
    _NC_CACHE = nc
    return nc


def _arrange_w(w):
    # [D, 128] -> on-chip [128, DC, 128] so the device DMA is dense
    return np.ascontiguousarray(w.reshape(DC, 128, 128).transpose(1, 0, 2))


def _pre_chunk(aT):
    # [D, 2048] (d_model-major transpose) -> [8, 128, 4, 512] pieces so each
    # partition's slice of a piece is 4 KB contiguous in DRAM
    t = aT.reshape(2, 4, 128, 4, 512)  # [h, c_local, p, ck, n]
    return np.ascontiguousarray(
        t.transpose(3, 0, 2, 1, 4).reshape(8, 128, 4, 512))


def make_in_maps(**inputs):
    bf16 = ml_dtypes.bfloat16
    enc = np.asarray(inputs["encoder_output"])
    dec = np.asarray(inputs["decoder"])
    wq1 = np.asarray(inputs["Wq"])
    wq_s = _arrange_w(np.concatenate([wq1, wq1], axis=1))
    wkv1 = _arrange_w(np.concatenate(
        [np.asarray(inputs["Wv"]), np.asarray(inputs["Wk"])], axis=1
    ))
    # [wkv | wq] packed on the DC axis
    w_all = np.concatenate([wkv1, wq_s], axis=1).astype(bf16)
    in_maps = []
    for c in range(N_CORES):
        b, kh = divmod(c, 2)
        in_maps.append({
            "encT": _pre_chunk(enc[b, kh * SKC:(kh + 1) * SKC, :].T.astype(bf16)),
            "decT": _pre_chunk(dec[b].T.astype(bf16)),
            "wkv": w_all,
        })
    return in_maps


def assemble(results):
    out = np.zeros((B, SQ, DIMS), np.float32)
    for b in range(B):
        o0 = results[2 * b]["out"].astype(np.float32)
        o1 = results[2 * b + 1]["out"].astype(np.float32)
        num = o0[0:DIMS] + o1[0:DIMS]
        den = o0[DIMS] + o1[DIMS]
        out[b] = (num / den).T
    return out


def kernel(**inputs) -> np.ndarray:
    nc = _build()
    in_maps = make_in_maps(**inputs)
    res = run_bass_kernel_spmd(nc, in_maps, core_ids=list(range(N_CORES)))
    return assemble(res.results)


# revision 15
# speedup vs baseline: 1.0217x; 1.0217x over previous
"""Cross-attention kernel for Trainium2, distributed over 8 NeuronCores.

Problem: B=4, Sk=4096, Sq=2048, d_model=1024, dims=64 (fp32 reference).

Sharding (hardcoded): core c -> (batch b = c//2, ENCODER half kh = c%2).
Each core computes partial-softmax attention of ALL 2048 decoder rows of its
batch against its 2048-key half of the encoder: a numerator [64, 2048] and a
denominator row accumulated in the same PSUM tile via a ones-column in the AV
lhsT. The host merges the two halves ((num0+num1)/(den0+den1)) and
transposes -- so the device does no softmax normalization, no output
transposes, and no collectives.

All compute is bf16: fp8 anywhere in the score/weight path (tested: at-fp8
alone 2.6e-2, v-fp8 1.9e-2, enc-fp8 4e-2 vs the 2e-2 gate) fails numerics,
so the PE floor is ~100k cycles/core and the kernel is PE-bound. The wins
over the original baseline are scheduling:
  - Flat 32-step software pipeline across both decoder halves (S of step g+1
    issues before AV of step g, across the qh boundary too).
  - DMA: wq/wkv issue from the Scalar queue in parallel with Sync; only the
    four first-needed transfers (enc0/dec0 halves) go on Sync up front; the
    six later activation chunks issue from GpSimd (SWDGE) after its memsets,
    so the critical first 2.5 MB gets the full ~390 GB/s instead of
    fair-sharing with 6 MB of later traffic.
  - gpsimd memset order: scratch (PE warmup input) first so warmup matmuls
    start right after the preamble barrier and the PE HAM clock-gate
    releases just as real work arrives; the big kTd/vTx memsets run on the
    (otherwise idle until ~13us) Vector engine instead of gpsimd.
  - Prologue projections consume enc0/dec0 half-by-half as they land; the
    kTd chunk-0 evacuation runs on ACT while qTd's runs on DVE in parallel.
  - exp's 1/sqrt(dims) score scale is folded into the ACT free affine
    (scale=0.125), so weights stream unscaled.
  - The softmax num/den go back to DRAM as bf16 (half the store tail); the
    host divides in fp32.
"""

import numpy as np
import ml_dtypes

import concourse.bass as bass
import concourse.bacc as bacc
import concourse.tile as tile
from concourse import mybir
from concourse._compat import with_exitstack
from concourse.bass_utils import run_bass_kernel_spmd
from concourse.masks import make_identity

BF16 = mybir.dt.bfloat16
F32 = mybir.dt.float32
B, SK, SQ, D, DIMS = 4, 4096, 2048, 1024, 64
N_CORES = 8
SKC = SK // 2   # 2048 encoder keys per core
SQC = SQ        # full decoder per core
DC = D // 128   # 8 d_model chunks
KB = SKC // 128  # 16 k blocks
NCK = SKC // 512  # 4 kv chunks
N_WARM = 8
EXP_SCALE = float(DIMS) ** -0.5  # 0.125, applied in the ACT free affine


@with_exitstack
def _body(ctx, tc, encT, decT, wkv, out):
    nc = tc.nc

    singles = ctx.enter_context(tc.tile_pool(name="singles", bufs=1))
    loads = ctx.enter_context(tc.tile_pool(name="loads", bufs=1))
    pss_pool = ctx.enter_context(tc.tile_pool(name="pss", bufs=2, space="PSUM"))
    po_pool = ctx.enter_context(tc.tile_pool(name="po", bufs=1, space="PSUM"))
    aux_pool = ctx.enter_context(tc.tile_pool(name="aux", bufs=2, space="PSUM"))
    at_pool = ctx.enter_context(tc.tile_pool(name="at", bufs=3))

    # --- scratch memset FIRST on gpsimd: it gates the PE warmup matmuls ---
    scratch = singles.tile([128, 512], BF16)
    nc.gpsimd.memset(scratch, 0.0)

    # --- weights on the Scalar queue: wq (first consumer) then wkv ---
    w_sb = singles.tile([128, 2 * DC, 128], BF16)
    nc.scalar.dma_start(out=w_sb[:, DC:2 * DC, :], in_=wkv[:, DC:2 * DC, :])
    nc.scalar.dma_start(out=w_sb[:, 0:DC, :], in_=wkv[:, 0:DC, :])
    wkv_sb = w_sb[:, 0:DC, :]
    wq_sb = w_sb[:, DC:2 * DC, :]

    # --- activation tiles ---
    esb = [
        loads.tile([128, DC, 512], BF16, tag=f"esb{ck}", name=f"esb{ck}")
        for ck in range(NCK)
    ]
    dsb = [
        loads.tile([128, DC, 512], BF16, tag=f"dsb{qg}", name=f"dsb{qg}")
        for qg in range(4)
    ]
    enc_r = encT  # [8, 128, 4, 512] pre-chunked on host
    dec_r = decT

    def load_enc(eng, ck):
        eng.dma_start(
            out=esb[ck].rearrange("p (h c) n -> p h c n", h=2),
            in_=enc_r[2 * ck:2 * ck + 2].rearrange("h p c n -> p h c n"),
        )

    def load_dec(eng, qg):
        eng.dma_start(
            out=dsb[qg].rearrange("p (h c) n -> p h c n", h=2),
            in_=dec_r[2 * qg:2 * qg + 2].rearrange("h p c n -> p h c n"),
        )

    # first-needed transfers on Sync, half-granular, arrival-need order
    nc.sync.dma_start(out=esb[0][:, 0:4, :], in_=enc_r[0])
    nc.sync.dma_start(out=dsb[0][:, 0:4, :], in_=dec_r[0])
    nc.sync.dma_start(out=esb[0][:, 4:8, :], in_=enc_r[1])
    nc.sync.dma_start(out=dsb[0][:, 4:8, :], in_=dec_r[1])

    # --- small constants on gpsimd, then the six later chunks via SWDGE so
    # they don't steal bandwidth from the four critical transfers above ---
    bv_sb = singles.tile([DIMS, 1], F32)
    nc.gpsimd.memset(bv_sb, 0.0)
    bk_sb = singles.tile([DIMS, 1], F32)
    nc.gpsimd.memset(bk_sb, 0.0)
    bq_sb = singles.tile([128, 1], F32)
    nc.gpsimd.memset(bq_sb, 0.0)
    ident_bf = singles.tile([128, 128], BF16)
    make_identity(nc, ident_bf)

    gate = singles.tile([1, 8], BF16)

    # --- persistent activations. The big zero/ones fills run on Vector
    # (idle until the first evacuation at ~13us); kTd rows 64:128 stay ZERO
    # so S matmuls run a full K=128 contraction ---
    kTd = singles.tile([128, SKC], BF16)
    nc.vector.memset(kTd[DIMS:128, :], 0.0)
    vTx = singles.tile([DIMS + 1, SKC], BF16)  # V^T (ones come via vnat)
    vnat = singles.tile([128, KB, 80], BF16)   # V natural + ones col 64
    nc.vector.memset(vnat[:, :, DIMS:DIMS + 1], 1.0)
    qTd = singles.tile([128, SQC], BF16)  # Q^T (unscaled) duplicated

    # --- PE warmup during the DMA prologue (HAM clock-gate release) ---
    wm = pss_pool.tile([128, 2, 512], F32, tag="pss", name="pss_w")
    for i in range(N_WARM):
        nc.tensor.matmul(
            wm[:, i % 2, :], lhsT=scratch[:, 0:128], rhs=scratch,
            start=True, stop=True,
        )

    # --- K/V projection per 512-column chunk: lhsT = [Wv | Wk], rhs = encT
    # chunk -> psum [128, 512], rows 0:64 = V^T, 64:128 = K^T ---
    kv_ps = {}

    def kv_mms(ck, lo, hi):
        if ck not in kv_ps:
            kv_ps[ck] = aux_pool.tile(
                [128, 512], F32, tag="aux", name=f"pskv{ck % 2}")
        for d in range(lo, hi):
            nc.tensor.matmul(
                kv_ps[ck], lhsT=wkv_sb[:, d, :], rhs=esb[ck][:, d, :],
                start=(d == 0), stop=(d == DC - 1),
            )

    def kv_evac(ck):
        # kTd first: it alone gates the next S pair (and thus the exp chain)
        pskv = kv_ps.pop(ck)
        sl = slice(ck * 512, (ck + 1) * 512)
        nc.vector.tensor_scalar_add(kTd[0:DIMS, sl], pskv[DIMS:128, :], bk_sb)
        nc.vector.tensor_scalar_add(vTx[0:DIMS, sl], pskv[0:DIMS, :], bv_sb)

    def kv_tr(ck, half):
        for kb in range(ck * 4 + 2 * half, ck * 4 + 2 * half + 2):
            ptv = aux_pool.tile([128, 80], BF16, tag="aux", name=f"ptv{kb % 2}")
            nc.tensor.transpose(
                ptv[:, 0:DIMS], vTx[0:DIMS, kb * 128:(kb + 1) * 128],
                ident_bf[0:DIMS, 0:DIMS],
            )
            nc.vector.tensor_copy(vnat[:, kb, 0:DIMS], ptv[:, 0:DIMS])



    qp_ps = {}

    def qproj_mms(qg, lo, hi):
        if qg not in qp_ps:
            qp_ps[qg] = aux_pool.tile(
                [128, 512], F32, tag="aux", name=f"psq{qg % 2}")
        for d in range(lo, hi):
            nc.tensor.matmul(
                qp_ps[qg], lhsT=wq_sb[:, d, :], rhs=dsb[qg][:, d, :],
                start=(d == 0), stop=(d == DC - 1),
            )

    def qproj_evac(qg):
        psq = qp_ps.pop(qg)
        nc.vector.tensor_scalar_add(qTd[:, qg * 512:(qg + 1) * 512], psq, bq_sb)

    # --- flat 32-step pipeline: step g -> (qh, kbp, sub) ---
    steps_local = [
        (2 * g + i, s)
        for g in range(KB // 4) for s in range(2) for i in range(2)
    ]
    NSTEP = 32
    at_tiles = {}

    def s_and_exp(gq):
        qh, (kbp, sub) = gq // 16, steps_local[gq % 16]
        pss = pss_pool.tile([128, 2, 512], F32, tag="pss", name=f"pss{gq % 2}")
        q0 = qh * 1024 + sub * 512
        for i in range(2):
            kb = 2 * kbp + i
            nc.tensor.matmul(
                pss[:, i, :], lhsT=kTd[:, kb * 128:(kb + 1) * 128],
                rhs=qTd[:, q0:q0 + 512],
                start=True, stop=True,
            )
        at = at_pool.tile([128, 2, 512], BF16, tag="at", name=f"at{gq % 3}")
        at_tiles[gq] = at
        nc.scalar.activation(
            at.rearrange("p a n -> p (a n)"),
            pss.rearrange("p a n -> p (a n)"),
            mybir.ActivationFunctionType.Exp,
            scale=EXP_SCALE,
        )

    def av(gq, po):
        qh, (kbp, sub) = gq // 16, steps_local[gq % 16]
        at = at_tiles.pop(gq)
        for i in range(2):
            nc.tensor.matmul(
                po[:, sub, :], lhsT=vnat[:, 2 * kbp + i, 0:DIMS + 1],
                rhs=at[:, i, :],
                start=(kbp == 0 and i == 0),
                stop=(kbp == KB // 2 - 1 and i == 1),
            )

    # --- prologue compute: projections of chunk 0 in DMA-half order; kTd
    # evac on the (idle) ACT engine, qTd evac on DVE in parallel ---
    ident_fn = mybir.ActivationFunctionType.Identity
    kv_mms(0, 0, 4)
    qproj_mms(0, 0, 4)
    kv_mms(0, 4, 8)
    qproj_mms(0, 4, 8)
    pskv0 = kv_ps.pop(0)
    # DMA deferral chain: each later activation transfer may only ISSUE once
    # the previous one has fully landed, so transfers run one-at-a-time at
    # full HBM bandwidth, in consumption order, starting only after the four
    # critical enc0/dec0 transfers are done. Program order alone does NOT
    # guarantee this (the tile scheduler reorders independent DMAs), so each
    # link is a real dependency: the tiny gpsimd add READS the previous
    # stream's last-landed cell (RAW on its DMA) and the next stream's tile
    # (WAR forces that DMA to wait for the add).
    chain_cells = [
        esb[0][0:1, 7, 511:512],
        dsb[1][0:1, 7, 511:512],
        esb[1][0:1, 7, 511:512],
        esb[2][0:1, 7, 511:512],
        esb[3][0:1, 7, 511:512],
        dsb[2][0:1, 7, 511:512],
        dsb[3][0:1, 7, 511:512],
    ]
    deferred = [
        lambda: load_dec(nc.gpsimd, 1),
        lambda: load_enc(nc.gpsimd, 1),
        lambda: load_enc(nc.gpsimd, 2),
        lambda: load_enc(nc.gpsimd, 3),
        lambda: load_dec(nc.gpsimd, 2),
        lambda: load_dec(nc.gpsimd, 3),
    ]
    for k, ld in enumerate(deferred):
        nc.gpsimd.tensor_tensor(
            gate[0:1, k:k + 1], chain_cells[k], chain_cells[k + 1],
            mybir.AluOpType.add,
        )
        ld()
    nc.scalar.activation(kTd[0:DIMS, 0:512], pskv0[DIMS:128, :], ident_fn,
                         bias=bk_sb)
    psq0 = qp_ps.pop(0)
    nc.vector.tensor_scalar_add(qTd[:, 0:512], psq0, bq_sb)
    nc.vector.tensor_scalar_add(vTx[0:DIMS, 0:512], pskv0[0:DIMS, :], bv_sb)

    # --- extra PE work injected at the steps its DMA has landed. kv chunk c
    # must be evacuated before the S-pair issue for step 4c (one-ahead at
    # step 4c-1, after that step's extras); vnat block pair from kv_tr(c, h)
    # before the AV of the step that consumes it. ---
    extras = {
        0: lambda: (kv_tr(0, 0), qproj_mms(1, 0, 4)),
        1: lambda: (kv_tr(0, 1), qproj_mms(1, 4, 8), qproj_evac(1)),
        2: lambda: kv_mms(1, 0, 4),
        3: lambda: (kv_mms(1, 4, 8), kv_evac(1)),
        4: lambda: kv_tr(1, 0),
        5: lambda: (kv_tr(1, 1), kv_mms(2, 0, 4)),
        6: lambda: (kv_mms(2, 4, 8), kv_evac(2)),
        8: lambda: kv_tr(2, 0),
        9: lambda: (kv_tr(2, 1), kv_mms(3, 0, 4)),
        10: lambda: (kv_mms(3, 4, 8), kv_evac(3)),
        12: lambda: kv_tr(3, 0),
        13: lambda: (kv_tr(3, 1), qproj_mms(2, 0, 4)),
        14: lambda: (qproj_mms(2, 4, 8), qproj_evac(2)),
        16: lambda: qproj_mms(3, 0, 4),
        17: lambda: (qproj_mms(3, 4, 8), qproj_evac(3)),
    }
    out_r = out.rearrange("p (h s n) -> p h s n", h=2, s=2)
    oT = singles.tile([DIMS + 1, SQC], BF16)
    oT_r = oT.rearrange("p (h s n) -> p h s n", h=2, s=2)

    pos = {}
    for gq in range(NSTEP):
        qh = gq // 16
        kbp, sub = steps_local[gq % 16]
        if gq == 0:
            s_and_exp(0)
        if gq in extras:
            extras[gq]()
        if gq + 1 < NSTEP:
            s_and_exp(gq + 1)
        if qh not in pos:
            pos[qh] = po_pool.tile([DIMS + 1, 2, 512], F32, tag="po", name="po")
        av(gq, pos[qh])
        if (kbp, sub) == (KB // 2 - 1, 0):
            # sub 0's accumulation completes two steps before sub 1's:
            # evacuate + store it under the remaining steps
            nc.vector.tensor_copy(oT_r[:, qh, 0, :], pos[qh][:, 0, :])
            nc.sync.dma_start(out=out_r[:, qh, 0, :], in_=oT_r[:, qh, 0, :])
        elif (kbp, sub) == (KB // 2 - 1, 1):
            nc.vector.tensor_copy(oT_r[:, qh, 1, :], pos[qh][:, 1, :])
            nc.sync.dma_start(out=out_r[:, qh, 1, :], in_=oT_r[:, qh, 1, :])


_NC_CACHE = None


def _build():
    global _NC_CACHE
    if _NC_CACHE is not None:
        return _NC_CACHE
    nc = bacc.Bacc(
        "TRN2", target_bir_lowering=False, debug=False,
        enable_asserts=True, num_devices=N_CORES,
    )
    encT = nc.dram_tensor(
        "encT", [2 * NCK, 128, 4, 512], BF16, kind="ExternalInput").ap()
    decT = nc.dram_tensor(
        "decT", [2 * 4, 128, 4, 512], BF16, kind="ExternalInput").ap()
    wkv = nc.dram_tensor(
        "wkv", [128, 2 * DC, 128], BF16, kind="ExternalInput").ap()
    out = nc.dram_tensor("out", [DIMS + 1, SQC], BF16, kind="ExternalOutput").ap()
    with tile.TileContext(nc) as tc:
        _body(tc, encT, decT, wkv, out)
    nc.compile()
    _NC_CACHE = nc
    return nc


def _arrange_w(w):
    # [D, 128] -> on-chip [128, DC, 128] so the device DMA is dense
    return np.ascontiguousarray(w.reshape(DC, 128, 128).transpose(1, 0, 2))


def _pre_chunk(aT):
    # [D, 2048] (d_model-major transpose) -> [8, 128, 4, 512] pieces so each
    # partition's slice of a piece is 4 KB contiguous in DRAM
    t = aT.reshape(2, 4, 128, 4, 512)  # [h, c_local, p, ck, n]
    return np.ascontiguousarray(
        t.transpose(3, 0, 2, 1, 4).reshape(8, 128, 4, 512))


def make_in_maps(**inputs):
    bf16 = ml_dtypes.bfloat16
    enc = np.asarray(inputs["encoder_output"])
    dec = np.asarray(inputs["decoder"])
    wq1 = np.asarray(inputs["Wq"])
    wq_s = _arrange_w(np.concatenate([wq1, wq1], axis=1))
    wkv1 = _arrange_w(np.concatenate(
        [np.asarray(inputs["Wv"]), np.asarray(inputs["Wk"])], axis=1
    ))
    # [wkv | wq] packed on the DC axis
    w_all = np.concatenate([wkv1, wq_s], axis=1).astype(bf16)
    in_maps = []
    for c in range(N_CORES):
        b, kh = divmod(c, 2)
        in_maps.append({
            "encT": _pre_chunk(enc[b, kh * SKC:(kh + 1) * SKC, :].T.astype(bf16)),
            "decT": _pre_chunk(dec[b].T.astype(bf16)),
            "wkv": w_all,
        })
    return in_maps


def assemble(results):
    out = np.zeros((B, SQ, DIMS), np.float32)
    for b in range(B):
        o0 = results[2 * b]["out"].astype(np.float32)
        o1 = results[2 * b + 1]["out"].astype(np.float32)
        num = o0[0:DIMS] + o1[0:DIMS]
        den = o0[DIMS] + o1[DIMS]
        out[b] = (num / den).T
    return out


def kernel(**inputs) -> np.ndarray:
    nc = _build()
    in_maps = make_in_maps(**inputs)
    res = run_bass_kernel_spmd(nc, in_maps, core_ids=list(range(N_CORES)))
    return assemble(res.results)


# revision 16
# speedup vs baseline: 1.1575x; 1.1329x over previous
"""Cross-attention kernel for Trainium2, distributed over 8 NeuronCores.

Problem: B=4, Sk=4096, Sq=2048, d_model=1024, dims=64 (fp32 reference).

Sharding (hardcoded): core c -> (batch b = c//2, ENCODER half kh = c%2).
Each core computes partial-softmax attention of ALL 2048 decoder rows of its
batch against its 2048-key half of the encoder: a numerator [64, 2048] and a
denominator row accumulated in the same PSUM tile via a ones-column in the AV
lhsT. The host merges the two halves ((num0+num1)/(den0+den1)) and
transposes -- so the device does no softmax normalization, no output
transposes, and no collectives.

All compute is bf16: fp8 anywhere in the score/weight path (tested: at-fp8
alone 2.6e-2, v-fp8 1.9e-2, enc-fp8 4e-2 vs the 2e-2 gate) fails numerics,
so the PE floor is ~100k cycles/core and the kernel is PE-bound. The wins
over the original baseline are scheduling:
  - Flat 32-step software pipeline across both decoder halves (S of step g+1
    issues before AV of step g, across the qh boundary too).
  - DMA: wq/wkv issue from the Scalar queue in parallel with Sync; only the
    four first-needed transfers (enc0/dec0 halves) go on Sync up front; the
    six later activation chunks issue from GpSimd (SWDGE) after its memsets,
    so the critical first 2.5 MB gets the full ~390 GB/s instead of
    fair-sharing with 6 MB of later traffic.
  - gpsimd memset order: scratch (PE warmup input) first so warmup matmuls
    start right after the preamble barrier and the PE HAM clock-gate
    releases just as real work arrives; the big kTd/vTx memsets run on the
    (otherwise idle until ~13us) Vector engine instead of gpsimd.
  - Prologue projections consume enc0/dec0 half-by-half as they land; the
    kTd chunk-0 evacuation runs on ACT while qTd's runs on DVE in parallel.
  - exp's 1/sqrt(dims) score scale is folded into the ACT free affine
    (scale=0.125), so weights stream unscaled.
  - The softmax num/den go back to DRAM as bf16 (half the store tail); the
    host divides in fp32.
"""

import numpy as np
import ml_dtypes

import concourse.bass as bass
import concourse.bacc as bacc
import concourse.tile as tile
from concourse import mybir
from concourse._compat import with_exitstack
from concourse.bass_utils import run_bass_kernel_spmd
from concourse.masks import make_identity

BF16 = mybir.dt.bfloat16
F32 = mybir.dt.float32
B, SK, SQ, D, DIMS = 4, 4096, 2048, 1024, 64
N_CORES = 8
SKC = SK // 2   # 2048 encoder keys per core
SQC = SQ        # full decoder per core
DC = D // 128   # 8 d_model chunks
KB = SKC // 128  # 16 k blocks
NCK = SKC // 512  # 4 kv chunks
N_WARM = 8
EXP_SCALE = float(DIMS) ** -0.5  # 0.125, applied in the ACT free affine


@with_exitstack
def _body(ctx, tc, encT, decT, wkv, out):
    nc = tc.nc

    singles = ctx.enter_context(tc.tile_pool(name="singles", bufs=1))
    loads = ctx.enter_context(tc.tile_pool(name="loads", bufs=1))
    pss_pool = ctx.enter_context(tc.tile_pool(name="pss", bufs=2, space="PSUM"))
    po_pool = ctx.enter_context(tc.tile_pool(name="po", bufs=1, space="PSUM"))
    aux_pool = ctx.enter_context(tc.tile_pool(name="aux", bufs=2, space="PSUM"))
    at_pool = ctx.enter_context(tc.tile_pool(name="at", bufs=3))

    # --- scratch memset FIRST on gpsimd: it gates the PE warmup matmuls ---
    scratch = singles.tile([128, 512], BF16)
    nc.gpsimd.memset(scratch, 0.0)

    # --- weights on the Scalar queue: wq (first consumer) then wkv ---
    w_sb = singles.tile([128, 2 * DC, 128], BF16)
    nc.scalar.dma_start(out=w_sb[:, DC:2 * DC, :], in_=wkv[:, DC:2 * DC, :])
    nc.scalar.dma_start(out=w_sb[:, 0:DC, :], in_=wkv[:, 0:DC, :])
    wkv_sb = w_sb[:, 0:DC, :]
    wq_sb = w_sb[:, DC:2 * DC, :]

    # --- activation tiles ---
    esb = [
        loads.tile([128, DC, 512], BF16, tag=f"esb{ck}", name=f"esb{ck}")
        for ck in range(NCK)
    ]
    dsb = [
        loads.tile([128, DC, 512], BF16, tag=f"dsb{qg}", name=f"dsb{qg}")
        for qg in range(4)
    ]
    enc_r = encT  # [8, 128, 4, 512] pre-chunked on host
    dec_r = decT

    def load_enc(eng, ck):
        eng.dma_start(
            out=esb[ck].rearrange("p (h c) n -> p h c n", h=2),
            in_=enc_r[2 * ck:2 * ck + 2].rearrange("h p c n -> p h c n"),
        )

    def load_dec(eng, qg):
        eng.dma_start(
            out=dsb[qg].rearrange("p (h c) n -> p h c n", h=2),
            in_=dec_r[2 * qg:2 * qg + 2].rearrange("h p c n -> p h c n"),
        )

    # first-needed transfers on Sync, half-granular, arrival-need order
    nc.sync.dma_start(out=esb[0][:, 0:4, :], in_=enc_r[0])
    nc.sync.dma_start(out=dsb[0][:, 0:4, :], in_=dec_r[0])
    nc.sync.dma_start(out=esb[0][:, 4:8, :], in_=enc_r[1])
    nc.sync.dma_start(out=dsb[0][:, 4:8, :], in_=dec_r[1])

    # --- small constants on gpsimd, then the six later chunks via SWDGE so
    # they don't steal bandwidth from the four critical transfers above ---
    bv_sb = singles.tile([DIMS, 1], F32)
    nc.gpsimd.memset(bv_sb, 0.0)
    bk_sb = singles.tile([DIMS, 1], F32)
    nc.gpsimd.memset(bk_sb, 0.0)
    bq_sb = singles.tile([128, 1], F32)
    nc.gpsimd.memset(bq_sb, 0.0)
    ident_bf = singles.tile([128, 128], BF16)
    make_identity(nc, ident_bf)

    gate = singles.tile([1, 8], BF16)

    # --- persistent activations. The big zero/ones fills run on Vector
    # (idle until the first evacuation at ~13us); kTd rows 64:128 stay ZERO
    # so S matmuls run a full K=128 contraction ---
    kTd = singles.tile([128, SKC], BF16)
    nc.vector.memset(kTd[DIMS:128, :], 0.0)
    vTx = singles.tile([DIMS + 1, SKC], BF16)  # V^T (ones come via vnat)
    vnat = singles.tile([128, KB, 80], BF16)   # V natural + ones col 64
    nc.vector.memset(vnat[:, :, DIMS:DIMS + 1], 1.0)
    qTd = singles.tile([128, SQC], BF16)  # Q^T (unscaled) duplicated

    # --- PE warmup during the DMA prologue (HAM clock-gate release) ---
    wm = pss_pool.tile([128, 2, 512], F32, tag="pss", name="pss_w")
    for i in range(N_WARM):
        nc.tensor.matmul(
            wm[:, i % 2, :], lhsT=scratch[:, 0:128], rhs=scratch,
            start=True, stop=True,
        )

    # --- K/V projection per 512-column chunk: lhsT = [Wv | Wk], rhs = encT
    # chunk -> psum [128, 512], rows 0:64 = V^T, 64:128 = K^T ---
    kv_ps = {}

    def kv_mms(ck, lo, hi):
        if ck not in kv_ps:
            kv_ps[ck] = aux_pool.tile(
                [128, 512], F32, tag="aux", name=f"pskv{ck % 2}")
        for d in range(lo, hi):
            nc.tensor.matmul(
                kv_ps[ck], lhsT=wkv_sb[:, d, :], rhs=esb[ck][:, d, :],
                start=(d == 0), stop=(d == DC - 1),
            )

    def kv_evac(ck):
        # kTd first: it alone gates the next S pair (and thus the exp chain)
        pskv = kv_ps.pop(ck)
        sl = slice(ck * 512, (ck + 1) * 512)
        nc.vector.tensor_scalar_add(kTd[0:DIMS, sl], pskv[DIMS:128, :], bk_sb)
        nc.vector.tensor_scalar_add(vTx[0:DIMS, sl], pskv[0:DIMS, :], bv_sb)

    def kv_tr(ck, half):
        for kb in range(ck * 4 + 2 * half, ck * 4 + 2 * half + 2):
            ptv = aux_pool.tile([128, 80], BF16, tag="aux", name=f"ptv{kb % 2}")
            nc.tensor.transpose(
                ptv[:, 0:DIMS], vTx[0:DIMS, kb * 128:(kb + 1) * 128],
                ident_bf[0:DIMS, 0:DIMS],
            )
            nc.vector.tensor_copy(vnat[:, kb, 0:DIMS], ptv[:, 0:DIMS])



    qp_ps = {}

    def qproj_mms(qg, lo, hi):
        if qg not in qp_ps:
            qp_ps[qg] = aux_pool.tile(
                [128, 512], F32, tag="aux", name=f"psq{qg % 2}")
        for d in range(lo, hi):
            nc.tensor.matmul(
                qp_ps[qg], lhsT=wq_sb[:, d, :], rhs=dsb[qg][:, d, :],
                start=(d == 0), stop=(d == DC - 1),
            )

    def qproj_evac(qg):
        psq = qp_ps.pop(qg)
        nc.vector.tensor_scalar_add(qTd[:, qg * 512:(qg + 1) * 512], psq, bq_sb)

    # --- flat 32-step pipeline: step g -> (qh, kbp, sub) ---
    steps_local = [
        (2 * g + i, s)
        for g in range(KB // 4) for s in range(2) for i in range(2)
    ]
    NSTEP = 32
    at_tiles = {}

    def s_and_exp(gq):
        qh, (kbp, sub) = gq // 16, steps_local[gq % 16]
        pss = pss_pool.tile([128, 2, 512], F32, tag="pss", name=f"pss{gq % 2}")
        q0 = qh * 1024 + sub * 512
        for i in range(2):
            kb = 2 * kbp + i
            nc.tensor.matmul(
                pss[:, i, :], lhsT=kTd[:, kb * 128:(kb + 1) * 128],
                rhs=qTd[:, q0:q0 + 512],
                start=True, stop=True,
            )
        at = at_pool.tile([128, 2, 512], BF16, tag="at", name=f"at{gq % 3}")
        at_tiles[gq] = at
        nc.scalar.activation(
            at.rearrange("p a n -> p (a n)"),
            pss.rearrange("p a n -> p (a n)"),
            mybir.ActivationFunctionType.Exp,
            scale=EXP_SCALE,
        )

    def av(gq, po):
        qh, (kbp, sub) = gq // 16, steps_local[gq % 16]
        at = at_tiles.pop(gq)
        for i in range(2):
            nc.tensor.matmul(
                po[:, sub, :], lhsT=vnat[:, 2 * kbp + i, 0:DIMS + 1],
                rhs=at[:, i, :],
                start=(kbp == 0 and i == 0),
                stop=(kbp == KB // 2 - 1 and i == 1),
            )

    # --- prologue compute: projections of chunk 0 in DMA-half order; kTd
    # evac on the (idle) ACT engine, qTd evac on DVE in parallel ---
    ident_fn = mybir.ActivationFunctionType.Identity
    kv_mms(0, 0, 4)
    qproj_mms(0, 0, 4)
    kv_mms(0, 4, 8)
    qproj_mms(0, 4, 8)
    pskv0 = kv_ps.pop(0)
    # DMA deferral chain, pipeline depth 2: transfer k may only ISSUE once
    # transfer k-2 has fully landed (and none before the four critical
    # enc0/dec0 transfers are done), keeping ~2 streams in flight at the
    # fabric's aggregate bandwidth without starving the critical prologue.
    # Program order alone does NOT guarantee any of this (the tile scheduler
    # reorders independent DMAs), so each link is a real dependency: the tiny
    # gpsimd add READS the (k-2)-stream's last-landed cell (RAW on its DMA)
    # and the k-stream's tile (WAR forces that DMA to wait for the add).
    d0_tail = dsb[0][0:1, 7, 511:512]
    cells = {
        "d1": dsb[1][0:1, 7, 511:512], "e1": esb[1][0:1, 7, 511:512],
        "e2": esb[2][0:1, 7, 511:512], "e3": esb[3][0:1, 7, 511:512],
        "d2": dsb[2][0:1, 7, 511:512], "d3": dsb[3][0:1, 7, 511:512],
    }
    deferred = [
        (d0_tail, "d1", lambda: load_dec(nc.gpsimd, 1)),
        (d0_tail, "e1", lambda: load_enc(nc.gpsimd, 1)),
        (cells["d1"], "e2", lambda: load_enc(nc.gpsimd, 2)),
        (cells["e1"], "e3", lambda: load_enc(nc.gpsimd, 3)),
        (cells["e2"], "d2", lambda: load_dec(nc.gpsimd, 2)),
        (cells["e3"], "d3", lambda: load_dec(nc.gpsimd, 3)),
    ]
    for k, (dep_cell, tgt, ld) in enumerate(deferred):
        nc.gpsimd.tensor_tensor(
            gate[0:1, k:k + 1], dep_cell, cells[tgt], mybir.AluOpType.add,
        )
        ld()
    nc.scalar.activation(kTd[0:DIMS, 0:512], pskv0[DIMS:128, :], ident_fn,
                         bias=bk_sb)
    psq0 = qp_ps.pop(0)
    nc.vector.tensor_scalar_add(qTd[:, 0:512], psq0, bq_sb)
    nc.vector.tensor_scalar_add(vTx[0:DIMS, 0:512], pskv0[0:DIMS, :], bv_sb)

    # --- extra PE work injected at the steps its DMA has landed. kv chunk c
    # must be evacuated before the S-pair issue for step 4c (one-ahead at
    # step 4c-1, after that step's extras); vnat block pair from kv_tr(c, h)
    # before the AV of the step that consumes it. ---
    extras = {
        0: lambda: (kv_tr(0, 0), qproj_mms(1, 0, 4)),
        1: lambda: (kv_tr(0, 1), qproj_mms(1, 4, 8), qproj_evac(1)),
        2: lambda: kv_mms(1, 0, 4),
        3: lambda: (kv_mms(1, 4, 8), kv_evac(1)),
        4: lambda: kv_tr(1, 0),
        5: lambda: (kv_tr(1, 1), kv_mms(2, 0, 4)),
        6: lambda: (kv_mms(2, 4, 8), kv_evac(2)),
        8: lambda: kv_tr(2, 0),
        9: lambda: (kv_tr(2, 1), kv_mms(3, 0, 4)),
        10: lambda: (kv_mms(3, 4, 8), kv_evac(3)),
        12: lambda: kv_tr(3, 0),
        13: lambda: (kv_tr(3, 1), qproj_mms(2, 0, 4)),
        14: lambda: (qproj_mms(2, 4, 8), qproj_evac(2)),
        16: lambda: qproj_mms(3, 0, 4),
        17: lambda: (qproj_mms(3, 4, 8), qproj_evac(3)),
    }
    out_r = out.rearrange("p (h s n) -> p h s n", h=2, s=2)
    oT = singles.tile([DIMS + 1, SQC], BF16)
    oT_r = oT.rearrange("p (h s n) -> p h s n", h=2, s=2)

    pos = {}
    for gq in range(NSTEP):
        qh = gq // 16
        kbp, sub = steps_local[gq % 16]
        if gq == 0:
            s_and_exp(0)
        if gq in extras:
            extras[gq]()
        if gq + 1 < NSTEP:
            s_and_exp(gq + 1)
        if qh not in pos:
            pos[qh] = po_pool.tile([DIMS + 1, 2, 512], F32, tag="po", name="po")
        av(gq, pos[qh])
        if (kbp, sub) == (KB // 2 - 1, 0):
            # sub 0's accumulation completes two steps before sub 1's:
            # evacuate + store it under the remaining steps
            nc.vector.tensor_copy(oT_r[:, qh, 0, :], pos[qh][:, 0, :])
            nc.sync.dma_start(out=out_r[:, qh, 0, :], in_=oT_r[:, qh, 0, :])
        elif (kbp, sub) == (KB // 2 - 1, 1):
            nc.vector.tensor_copy(oT_r[:, qh, 1, :], pos[qh][:, 1, :])
            nc.sync.dma_start(out=out_r[:, qh, 1, :], in_=oT_r[:, qh, 1, :])


_NC_CACHE = None


def _build():
    global _NC_CACHE
    if _NC_CACHE is not None:
        return _NC_CACHE
    nc = bacc.Bacc(
        "TRN2", target_bir_lowering=False, debug=False,
        enable_asserts=True, num_devices=N_CORES,
    )
    encT = nc.dram_tensor(
        "encT", [2 * NCK, 128, 4, 512], BF16, kind="ExternalInput").ap()
    decT = nc.dram_tensor(
        "decT", [2 * 4, 128, 4, 512], BF16, kind="ExternalInput").ap()
    wkv = nc.dram_tensor(
        "wkv", [128, 2 * DC, 128], BF16, kind="ExternalInput").ap()
    out = nc.dram_tensor("out", [DIMS + 1, SQC], BF16, kind="ExternalOutput").ap()
    with tile.TileContext(nc) as tc:
        _body(tc, encT, decT, wkv, out)
    nc.compile()
    _NC_CACHE = nc
    return nc


def _arrange_w(w):
    # [D, 128] -> on-chip [128, DC, 128] so the device DMA is dense
    return np.ascontiguousarray(w.reshape(DC, 128, 128).transpose(1, 0, 2))


def _pre_chunk(aT):
    # [D, 2048] (d_model-major transpose) -> [8, 128, 4, 512] pieces so each
    # partition's slice of a piece is 4 KB contiguous in DRAM
    t = aT.reshape(2, 4, 128, 4, 512)  # [h, c_local, p, ck, n]
    return np.ascontiguousarray(
        t.transpose(3, 0, 2, 1, 4).reshape(8, 128, 4, 512))


def make_in_maps(**inputs):
    bf16 = ml_dtypes.bfloat16
    enc = np.asarray(inputs["encoder_output"])
    dec = np.asarray(inputs["decoder"])
    wq1 = np.asarray(inputs["Wq"])
    wq_s = _arrange_w(np.concatenate([wq1, wq1], axis=1))
    wkv1 = _arrange_w(np.concatenate(
        [np.asarray(inputs["Wv"]), np.asarray(inputs["Wk"])], axis=1
    ))
    # [wkv | wq] packed on the DC axis
    w_all = np.concatenate([wkv1, wq_s], axis=1).astype(bf16)
    in_maps = []
    for c in range(N_CORES):
        b, kh = divmod(c, 2)
        in_maps.append({
            "encT": _pre_chunk(enc[b, kh * SKC:(kh + 1) * SKC, :].T.astype(bf16)),
            "decT": _pre_chunk(dec[b].T.astype(bf16)),
            "wkv": w_all,
        })
    return in_maps


def assemble(results):
    out = np.zeros((B, SQ, DIMS), np.float32)
    for b in range(B):
        o0 = results[2 * b]["out"].astype(np.float32)
        o1 = results[2 * b + 1]["out"].astype(np.float32)
        num = o0[0:DIMS] + o1[0:DIMS]
        den = o0[DIMS] + o1[DIMS]
        out[b] = (num / den).T
    return out


def kernel(**inputs) -> np.ndarray:
    nc = _build()
    in_maps = make_in_maps(**inputs)
    res = run_bass_kernel_spmd(nc, in_maps, core_ids=list(range(N_CORES)))
    return assemble(res.results)


# revision 20
# speedup vs baseline: 1.1902x; 1.0282x over previous
"""Cross-attention kernel for Trainium2, distributed over 8 NeuronCores.

Problem: B=4, Sk=4096, Sq=2048, d_model=1024, dims=64 (fp32 reference).

Sharding (hardcoded): core c -> (batch b = c//2, ENCODER half kh = c%2).
Each core computes partial-softmax attention of ALL 2048 decoder rows of its
batch against its 2048-key half of the encoder: a numerator [64, 2048] and a
denominator row accumulated in the same PSUM tile via a ones-column in the AV
lhsT. The host merges the two halves ((num0+num1)/(den0+den1)) and
transposes -- so the device does no softmax normalization, no output
transposes, and no collectives.

All compute is bf16: fp8 anywhere in the score/weight path (tested: at-fp8
alone 2.6e-2, v-fp8 1.9e-2, enc-fp8 4e-2 vs the 2e-2 gate) fails numerics,
so the PE floor is ~100k cycles/core and the kernel is PE-bound. The wins
over the original baseline are scheduling:
  - Flat 32-step software pipeline across both decoder halves (S of step g+1
    issues before AV of step g, across the qh boundary too).
  - DMA: wq/wkv issue from the Scalar queue in parallel with Sync; only the
    four first-needed transfers (enc0/dec0 halves) go on Sync up front; the
    six later activation chunks issue from GpSimd (SWDGE) after its memsets,
    so the critical first 2.5 MB gets the full ~390 GB/s instead of
    fair-sharing with 6 MB of later traffic.
  - gpsimd memset order: scratch (PE warmup input) first so warmup matmuls
    start right after the preamble barrier and the PE HAM clock-gate
    releases just as real work arrives; the big kTd/vTx memsets run on the
    (otherwise idle until ~13us) Vector engine instead of gpsimd.
  - Prologue projections consume enc0/dec0 half-by-half as they land; the
    kTd chunk-0 evacuation runs on ACT while qTd's runs on DVE in parallel.
  - exp's 1/sqrt(dims) score scale is folded into the ACT free affine
    (scale=0.125), so weights stream unscaled.
  - The softmax num/den go back to DRAM as bf16 (half the store tail); the
    host divides in fp32.
"""

import numpy as np
import ml_dtypes

import concourse.bass as bass
import concourse.bacc as bacc
import concourse.tile as tile
from concourse import mybir
from concourse._compat import with_exitstack
from concourse.bass_utils import run_bass_kernel_spmd
from concourse.masks import make_identity

BF16 = mybir.dt.bfloat16
F32 = mybir.dt.float32
B, SK, SQ, D, DIMS = 4, 4096, 2048, 1024, 64
N_CORES = 8
SKC = SK // 2   # 2048 encoder keys per core
SQC = SQ        # full decoder per core
DC = D // 128   # 8 d_model chunks
KB = SKC // 128  # 16 k blocks
NCK = SKC // 512  # 4 kv chunks
N_WARM = 8
EXP_SCALE = float(DIMS) ** -0.5  # 0.125, applied in the ACT free affine


@with_exitstack
def _body(ctx, tc, encT, decT, wkv, out):
    nc = tc.nc

    singles = ctx.enter_context(tc.tile_pool(name="singles", bufs=1))
    loads = ctx.enter_context(tc.tile_pool(name="loads", bufs=1))
    pss_pool = ctx.enter_context(tc.tile_pool(name="pss", bufs=2, space="PSUM"))
    po_pool = ctx.enter_context(tc.tile_pool(name="po", bufs=1, space="PSUM"))
    aux_pool = ctx.enter_context(tc.tile_pool(name="aux", bufs=2, space="PSUM"))
    at_pool = ctx.enter_context(tc.tile_pool(name="at", bufs=3))

    # --- scratch memset FIRST on gpsimd: it gates the PE warmup matmuls ---
    scratch = singles.tile([128, 512], BF16)
    nc.gpsimd.memset(scratch, 0.0)

    # --- weights in one dense DMA on the Scalar queue (parallel to Sync's
    # activation stream; 4 KB/partition contiguous so it runs at line rate) ---
    w_sb = singles.tile([128, 2 * DC, 128], BF16)
    nc.scalar.dma_start(out=w_sb, in_=wkv)
    wkv_sb = w_sb[:, 0:DC, :]
    wq_sb = w_sb[:, DC:2 * DC, :]

    # --- activation tiles ---
    esb = [
        loads.tile([128, DC, 512], BF16, tag=f"esb{ck}", name=f"esb{ck}")
        for ck in range(NCK)
    ]
    dsb = [
        loads.tile([128, DC, 512], BF16, tag=f"dsb{qg}", name=f"dsb{qg}")
        for qg in range(4)
    ]
    enc_r = encT  # [8, 128, 4, 512] pre-chunked on host
    dec_r = decT

    def load_enc(eng, ck):
        eng.dma_start(
            out=esb[ck].rearrange("p (h c) n -> p h c n", h=2),
            in_=enc_r[2 * ck:2 * ck + 2].rearrange("h p c n -> p h c n"),
        )

    def load_dec(eng, qg):
        eng.dma_start(
            out=dsb[qg].rearrange("p (h c) n -> p h c n", h=2),
            in_=dec_r[2 * qg:2 * qg + 2].rearrange("h p c n -> p h c n"),
        )

    # all activation loads on Sync in consumption order; enc0/dec0 split in
    # halves so projection pairs start after the first 512 KB lands. The
    # serial ~0.7us issue cost per DMA is the only stagger -- measured, any
    # stronger serialization (completion-gated chains) loses to the fabric's
    # fair-share behavior.
    nc.sync.dma_start(out=esb[0][:, 0:4, :], in_=enc_r[0])
    nc.sync.dma_start(out=dsb[0][:, 0:4, :], in_=dec_r[0])
    nc.sync.dma_start(out=esb[0][:, 4:8, :], in_=enc_r[1])
    nc.sync.dma_start(out=dsb[0][:, 4:8, :], in_=dec_r[1])
    load_dec(nc.sync, 1)
    load_enc(nc.sync, 1)
    load_enc(nc.sync, 2)
    load_enc(nc.sync, 3)
    load_dec(nc.sync, 2)
    load_dec(nc.sync, 3)

    # --- small constants on gpsimd, then the six later chunks via SWDGE so
    # they don't steal bandwidth from the four critical transfers above ---
    bv_sb = singles.tile([DIMS, 1], F32)
    nc.gpsimd.memset(bv_sb, 0.0)
    bk_sb = singles.tile([DIMS, 1], F32)
    nc.gpsimd.memset(bk_sb, 0.0)
    bq_sb = singles.tile([128, 1], F32)
    nc.gpsimd.memset(bq_sb, 0.0)
    ident_bf = singles.tile([128, 128], BF16)
    make_identity(nc, ident_bf)



    # --- persistent activations. The big zero/ones fills run on Vector
    # (idle until the first evacuation at ~13us); kTd rows 64:128 stay ZERO
    # so S matmuls run a full K=128 contraction ---
    kTd = singles.tile([128, SKC], BF16)
    nc.vector.memset(kTd[DIMS:128, :], 0.0)
    vTx = singles.tile([DIMS + 1, SKC], BF16)  # V^T (ones come via vnat)
    vnat = singles.tile([128, KB, 80], BF16)   # V natural + ones col 64
    nc.vector.memset(vnat[:, :, DIMS:DIMS + 1], 1.0)
    qTd = singles.tile([128, SQC], BF16)  # Q^T (unscaled) duplicated

    # --- PE warmup during the DMA prologue (HAM clock-gate release) ---
    wm = pss_pool.tile([128, 2, 512], F32, tag="pss", name="pss_w")
    for i in range(N_WARM):
        nc.tensor.matmul(
            wm[:, i % 2, :], lhsT=scratch[:, 0:128], rhs=scratch,
            start=True, stop=True,
        )

    # --- K/V projection per 512-column chunk: lhsT = [Wv | Wk], rhs = encT
    # chunk -> psum [128, 512], rows 0:64 = V^T, 64:128 = K^T ---
    kv_ps = {}

    def kv_mms(ck, lo, hi):
        if ck not in kv_ps:
            kv_ps[ck] = aux_pool.tile(
                [128, 512], F32, tag="aux", name=f"pskv{ck % 2}")
        for d in range(lo, hi):
            nc.tensor.matmul(
                kv_ps[ck], lhsT=wkv_sb[:, d, :], rhs=esb[ck][:, d, :],
                start=(d == 0), stop=(d == DC - 1),
            )

    def kv_evac(ck):
        # kTd first: it alone gates the next S pair (and thus the exp chain)
        pskv = kv_ps.pop(ck)
        sl = slice(ck * 512, (ck + 1) * 512)
        nc.vector.tensor_scalar_add(kTd[0:DIMS, sl], pskv[DIMS:128, :], bk_sb)
        nc.vector.tensor_scalar_add(vTx[0:DIMS, sl], pskv[0:DIMS, :], bv_sb)

    def kv_tr(ck, half):
        for kb in range(ck * 4 + 2 * half, ck * 4 + 2 * half + 2):
            ptv = aux_pool.tile([128, 80], BF16, tag="aux", name=f"ptv{kb % 2}")
            nc.tensor.transpose(
                ptv[:, 0:DIMS], vTx[0:DIMS, kb * 128:(kb + 1) * 128],
                ident_bf[0:DIMS, 0:DIMS],
            )
            nc.vector.tensor_copy(vnat[:, kb, 0:DIMS], ptv[:, 0:DIMS])



    qp_ps = {}

    def qproj_mms(qg, lo, hi):
        if qg not in qp_ps:
            qp_ps[qg] = aux_pool.tile(
                [128, 512], F32, tag="aux", name=f"psq{qg % 2}")
        for d in range(lo, hi):
            nc.tensor.matmul(
                qp_ps[qg], lhsT=wq_sb[:, d, :], rhs=dsb[qg][:, d, :],
                start=(d == 0), stop=(d == DC - 1),
            )

    def qproj_evac(qg):
        psq = qp_ps.pop(qg)
        nc.vector.tensor_scalar_add(qTd[:, qg * 512:(qg + 1) * 512], psq, bq_sb)

    # --- flat 32-step pipeline: step g -> (qh, kbp, sub) ---
    steps_local = [
        (2 * g + i, s)
        for g in range(KB // 4) for s in range(2) for i in range(2)
    ]
    NSTEP = 32
    at_tiles = {}

    def s_and_exp(gq):
        qh, (kbp, sub) = gq // 16, steps_local[gq % 16]
        pss = pss_pool.tile([128, 2, 512], F32, tag="pss", name=f"pss{gq % 2}")
        q0 = qh * 1024 + sub * 512
        for i in range(2):
            kb = 2 * kbp + i
            nc.tensor.matmul(
                pss[:, i, :], lhsT=kTd[:, kb * 128:(kb + 1) * 128],
                rhs=qTd[:, q0:q0 + 512],
                start=True, stop=True,
            )
        at = at_pool.tile([128, 2, 512], BF16, tag="at", name=f"at{gq % 3}")
        at_tiles[gq] = at
        nc.scalar.activation(
            at.rearrange("p a n -> p (a n)"),
            pss.rearrange("p a n -> p (a n)"),
            mybir.ActivationFunctionType.Exp,
            scale=EXP_SCALE,
        )

    def av(gq, po):
        qh, (kbp, sub) = gq // 16, steps_local[gq % 16]
        at = at_tiles.pop(gq)
        for i in range(2):
            nc.tensor.matmul(
                po[:, sub, :], lhsT=vnat[:, 2 * kbp + i, 0:DIMS + 1],
                rhs=at[:, i, :],
                start=(kbp == 0 and i == 0),
                stop=(kbp == KB // 2 - 1 and i == 1),
            )

    # --- prologue compute: projections of chunk 0 in DMA-half order; kTd
    # evac on the (idle) ACT engine, qTd evac on DVE in parallel ---
    ident_fn = mybir.ActivationFunctionType.Identity
    kv_mms(0, 0, 4)
    qproj_mms(0, 0, 4)
    kv_mms(0, 4, 8)
    qproj_mms(0, 4, 8)
    pskv0 = kv_ps.pop(0)
    nc.scalar.activation(kTd[0:DIMS, 0:512], pskv0[DIMS:128, :], ident_fn,
                         bias=bk_sb)
    psq0 = qp_ps.pop(0)
    nc.vector.tensor_scalar_add(qTd[:, 0:512], psq0, bq_sb)
    nc.vector.tensor_scalar_add(vTx[0:DIMS, 0:512], pskv0[0:DIMS, :], bv_sb)

    # --- extra PE work injected at the steps its DMA has landed. kv chunk c
    # must be evacuated before the S-pair issue for step 4c (one-ahead at
    # step 4c-1, after that step's extras); vnat block pair from kv_tr(c, h)
    # before the AV of the step that consumes it. ---
    extras = {
        0: lambda: (kv_tr(0, 0), qproj_mms(1, 0, 4)),
        1: lambda: (kv_tr(0, 1), qproj_mms(1, 4, 8), qproj_evac(1)),
        2: lambda: kv_mms(1, 0, 4),
        3: lambda: (kv_mms(1, 4, 8), kv_evac(1)),
        4: lambda: kv_tr(1, 0),
        5: lambda: (kv_tr(1, 1), kv_mms(2, 0, 4)),
        6: lambda: (kv_mms(2, 4, 8), kv_evac(2)),
        8: lambda: kv_tr(2, 0),
        9: lambda: (kv_tr(2, 1), kv_mms(3, 0, 4)),
        10: lambda: (kv_mms(3, 4, 8), kv_evac(3)),
        12: lambda: kv_tr(3, 0),
        13: lambda: (kv_tr(3, 1), qproj_mms(2, 0, 4)),
        14: lambda: (qproj_mms(2, 4, 8), qproj_evac(2)),
        16: lambda: qproj_mms(3, 0, 4),
        17: lambda: (qproj_mms(3, 4, 8), qproj_evac(3)),
    }
    out_r = out.rearrange("p (h s n) -> p h s n", h=2, s=2)
    oT = singles.tile([DIMS + 1, SQC], BF16)
    oT_r = oT.rearrange("p (h s n) -> p h s n", h=2, s=2)

    pos = {}
    for gq in range(NSTEP):
        qh = gq // 16
        kbp, sub = steps_local[gq % 16]
        if gq == 0:
            s_and_exp(0)
        if gq in extras:
            extras[gq]()
        if gq + 1 < NSTEP:
            s_and_exp(gq + 1)
        if qh not in pos:
            pos[qh] = po_pool.tile([DIMS + 1, 2, 512], F32, tag="po", name="po")
        av(gq, pos[qh])
        if (kbp, sub) == (KB // 2 - 1, 0):
            # sub 0's accumulation completes two steps before sub 1's:
            # evacuate + store it under the remaining steps
            nc.vector.tensor_copy(oT_r[:, qh, 0, :], pos[qh][:, 0, :])
            nc.sync.dma_start(out=out_r[:, qh, 0, :], in_=oT_r[:, qh, 0, :])
        elif (kbp, sub) == (KB // 2 - 1, 1):
            nc.vector.tensor_copy(oT_r[:, qh, 1, :], pos[qh][:, 1, :])
            nc.sync.dma_start(out=out_r[:, qh, 1, :], in_=oT_r[:, qh, 1, :])


_NC_CACHE = None


def _build():
    global _NC_CACHE
    if _NC_CACHE is not None:
        return _NC_CACHE
    nc = bacc.Bacc(
        "TRN2", target_bir_lowering=False, debug=False,
        enable_asserts=True, num_devices=N_CORES,
    )
    encT = nc.dram_tensor(
        "encT", [2 * NCK, 128, 4, 512], BF16, kind="ExternalInput").ap()
    decT = nc.dram_tensor(
        "decT", [2 * 4, 128, 4, 512], BF16, kind="ExternalInput").ap()
    wkv = nc.dram_tensor(
        "wkv", [128, 2 * DC, 128], BF16, kind="ExternalInput").ap()
    out = nc.dram_tensor("out", [DIMS + 1, SQC], BF16, kind="ExternalOutput").ap()
    with tile.TileContext(nc) as tc:
        _body(tc, encT, decT, wkv, out)
    nc.compile()
    _NC_CACHE = nc
    return nc


def _arrange_w(w):
    # [D, 128] -> on-chip [128, DC, 128] so the device DMA is dense
    return np.ascontiguousarray(w.reshape(DC, 128, 128).transpose(1, 0, 2))


def _pre_chunk(aT):
    # [D, 2048] (d_model-major transpose) -> [8, 128, 4, 512] pieces so each
    # partition's slice of a piece is 4 KB contiguous in DRAM
    t = aT.reshape(2, 4, 128, 4, 512)  # [h, c_local, p, ck, n]
    return np.ascontiguousarray(
        t.transpose(3, 0, 2, 1, 4).reshape(8, 128, 4, 512))


def make_in_maps(**inputs):
    bf16 = ml_dtypes.bfloat16
    enc = np.asarray(inputs["encoder_output"])
    dec = np.asarray(inputs["decoder"])
    wq1 = np.asarray(inputs["Wq"])
    wq_s = _arrange_w(np.concatenate([wq1, wq1], axis=1))
    wkv1 = _arrange_w(np.concatenate(
        [np.asarray(inputs["Wv"]), np.asarray(inputs["Wk"])], axis=1
    ))
    # [wkv | wq] packed on the DC axis
    w_all = np.concatenate([wkv1, wq_s], axis=1).astype(bf16)
    in_maps = []
    for c in range(N_CORES):
        b, kh = divmod(c, 2)
        in_maps.append({
            "encT": _pre_chunk(enc[b, kh * SKC:(kh + 1) * SKC, :].T.astype(bf16)),
            "decT": _pre_chunk(dec[b].T.astype(bf16)),
            "wkv": w_all,
        })
    return in_maps


def assemble(results):
    out = np.zeros((B, SQ, DIMS), np.float32)
    for b in range(B):
        o0 = results[2 * b]["out"].astype(np.float32)
        o1 = results[2 * b + 1]["out"].astype(np.float32)
        num = o0[0:DIMS] + o1[0:DIMS]
        den = o0[DIMS] + o1[DIMS]
        out[b] = (num / den).T
    return out


def kernel(**inputs) -> np.ndarray:
    nc = _build()
    in_maps = make_in_maps(**inputs)
    res = run_bass_kernel_spmd(nc, in_maps, core_ids=list(range(N_CORES)))
    return assemble(res.results)
